# revision 3
# baseline (speedup 1.0000x reference)
"""CrossDomainInterestLoss on 8 Trainium2 NeuronCores.

Strategy (hardcoded for bs=4096, dim=128):
  - Host: l2-normalize u/a/b (fp32), pre-transpose to [dim, rows] so the
    device needs no transposes, shard rows of u 4-way and cols (negatives)
    2-way -> 8 cores in a 4x2 grid.
  - Device (SPMD, identical program): for each 128-row chunk of u and each
    negatives matrix m in {A, B}:
      PE:  sim chunk [128, 2048] = uT_chunk.T @ mT  (4 matmuls into 4 PSUM banks)
      ACT: exp(sim/tau) with fused accum_out -> per-row partial pos sums
      DVE: relu(sim - margin) = (sim max m) add -m, fused accum -> relu sums
      DVE: is_gt(relu_tile, 0) on bf16 (4x mode), fused accum -> counts
    A few relu instructions run on ACT instead of DVE for engine balance.
  - Host: sum shard partials per row, analytically remove the diagonal
    contribution (device sums include j == i), then apply the exact
    reference formula on [4096] vectors.
"""

import numpy as np

import concourse.bass as bass
import concourse.mybir as mybir
from concourse import bacc, tile
from concourse import dve_ops as _dve_ops
from concourse.bass_utils import run_bass_kernel_spmd
from concourse.dve_ops import DveOp
from concourse.dve_spec import C0, C1, Spec, Src0, Zero, lower, relu, select
from concourse.dve_uop import DveOpSpec

TAU = 0.05
HARD_NEG_WEIGHT = 0.5
MARGIN = 0.3
BS = 4096
DIM = 128

R, C = 4, 2           # row-groups x col-groups = 8 cores
ROWS = BS // R        # u rows per core
COLS = BS // C        # negative rows per core (per matrix)
NRC = ROWS // 128     # 128-row chunks per core
NMM = COLS // 512     # matmuls per chunk per matrix

F32 = mybir.dt.float32
F32R = mybir.dt.float32r
BF16 = mybir.dt.bfloat16

# Custom DVE op: one pass over sim computing
#   out = relu(x - C0) + C1 * (x > C0),  accum = sum(out)
# With C1 = PACK_C the per-row accum packs both HNM quantities:
#   accum = relu_sum + PACK_C * count   (count <= ~30 per row here, so
#   PACK_C * count stays ~2^14 and fp32 accum keeps relu_sum precision).
PACK_C = 512.0


def _ref_relu_cnt_pack(in0, in1, s0, s1, imm2):
    r = np.maximum(in0.astype(np.float32) - s0, 0).astype(np.float32)
    g = ((in0 > s0).astype(np.float32) * s1).astype(np.float32)
    b = (r + g).astype(np.float32)
    return b, b.reshape(b.shape[0], -1).sum(axis=-1, keepdims=True).astype(np.float32)


def _get_packed_op():
    from operator import add as _add

    name = "RELU_CNT_PACK_ANT"
    for op in _dve_ops.OPS:
        if op.name == name:
            return op
    spec = Spec(
        body=relu(Src0 - C0) + select(Src0 > C0, C1, Zero),
        accum=_add,
        accum_init=Zero,
        reference=_ref_relu_cnt_pack,
    )
    row = _dve_ops._CUSTOM_DVE_ROW_BASE + len(_dve_ops.OPS)
    assert row < 0x20
    shas = {}
    for ver in ("v3", "v4"):
        try:
            uops = lower(spec, ver=ver)
            shas[ver] = DveOpSpec(
                name=name, opcode=row, uops=uops, rd1_en=False
            ).sha(ver)
        except Exception:
            pass
    op = DveOp(name, spec, subdim=False, uops_sha=shas)
    _dve_ops.OPS.append(op)
    _dve_ops._SUB_OPCODE_FOR_NAME[name] = row
    _dve_ops.CUSTOM_DVE_SPECS[name] = spec
    return op

# (rc, m) pairs whose relu+accum runs on ACT instead of DVE (engine balance)
ACT_RELU_PAIRS = set()

# PSUM group width: 1024 -> 2 banks x 4 bufs, 2048 -> 4 banks x 2 bufs.
GROUP_COLS = 1024
NG = COLS // GROUP_COLS  # accum columns per (rc, m)
NMM_G = GROUP_COLS // 512
PSUM_BUFS = 8192 // GROUP_COLS // 2

_BUILT = None
LAST_RESULTS = None  # BassKernelResults of the last run (for profiling)
TRACE = False
REPS = 1  # unrolled repetitions of the whole compute (wall-clock slope timing)
DYN_REPS = 0  # if > 0, wrap the compute in a For_i with this trip count


def _build_bass():
    global PACKED_OP
    PACKED_OP = _get_packed_op()
    nc = bacc.Bacc()

    # float32r: fp32 pre-rounded on the host to the PE's two-bf16 split so
    # matmuls stream at 1 cyc/col instead of fp32's 4.
    ut = nc.dram_tensor("ut", [DIM, ROWS], F32R, kind="ExternalInput")
    at = nc.dram_tensor("at", [DIM, COLS], F32R, kind="ExternalInput")
    bt = nc.dram_tensor("bt", [DIM, COLS], F32R, kind="ExternalInput")

    outs = {}
    for name in ("pos_a", "pos_b", "rsum_a", "rsum_b", "cnt_a", "cnt_b"):
        outs[name] = nc.dram_tensor(
            name, [128, NRC * NG], F32, kind="ExternalOutput"
        )

    with tile.TileContext(nc) as tc:
        with (
            tc.tile_pool(name="ops", bufs=1) as ops,
            tc.tile_pool(name="stats", bufs=1) as stats,
            tc.tile_pool(name="escr", bufs=2) as escr,
            tc.tile_pool(name="rscr", bufs=2) as rscr,
            tc.tile_pool(name="gscr", bufs=2) as gscr,
            tc.tile_pool(
                name="psum", bufs=PSUM_BUFS, space=bass.MemorySpace.PSUM
            ) as psum,
        ):
            ut_s = ops.tile([DIM, ROWS], F32R, tag="ut")
            at_s = ops.tile([DIM, COLS], F32R, tag="at")
            bt_s = ops.tile([DIM, COLS], F32R, tag="bt")
            # Loads split across the SP HWDGE path and the gpsimd SWDGE path
            # so ut and the first at half land in parallel and the first
            # matmul starts ~2.8us in.
            half = COLS // 2
            nc.gpsimd.dma_start(ut_s[:], ut[:])
            nc.sync.dma_start(at_s[:, :512], at[:, :512])
            nc.sync.dma_start(at_s[:, 512:half], at[:, 512:half])
            nc.sync.dma_start(at_s[:, half:], at[:, half:])
            nc.sync.dma_start(bt_s[:, :half], bt[:, :half])
            nc.sync.dma_start(bt_s[:, half:], bt[:, half:])

            st = {
                n: stats.tile([128, NRC * NG], F32, tag=n, name=n) for n in outs
            }
            # Dummy 1-element exp as the first ACT instruction: the compiler
            # inserts LoadActFuncSet right before it, so the ~1.3us table
            # load overlaps the input DMAs instead of the first real exp.
            warm = stats.tile([128, 1], F32, tag="warm", name="warm")
            nc.scalar.activation(
                warm[:],
                nc.const_aps.tensor(0.0, (128, 1), F32),
                mybir.ActivationFunctionType.Exp,
            )
            neg_margin = stats.tile([128, 1], F32, tag="neg_margin")
            nc.gpsimd.memset(neg_margin[:], -MARGIN)
            # Zero stats so columns never written on device (cnt under the
            # packed op; odd columns in RING_MODE) read as 0.
            for n in outs:
                nc.gpsimd.memset(st[n][:], 0.0)
            neg = {0: at_s, 1: bt_s}
            sfx = {0: "a", 1: "b"}

            def emit_mm(lhsT, m, g):
                sim = psum.tile([128, GROUP_COLS], F32, tag="sim", name="sim")
                for n in range(NMM_G):
                    j0 = g * GROUP_COLS + n * 512
                    nc.tensor.matmul(
                        sim[:, n * 512 : (n + 1) * 512],
                        lhsT,
                        neg[m][:, j0 : j0 + 512],
                        start=True,
                        stop=True,
                    )
                return sim

            def emit_exp(rc, m, sim, g):
                # exp(sim/tau), fused fp32 row-sum -> pos partials; the bf16
                # out tile itself is unused.
                col = slice(rc * NG + g, rc * NG + g + 1)
                e_t = escr.tile([128, GROUP_COLS], BF16, tag="e", name="e")
                nc.scalar.activation(
                    e_t[:],
                    sim[:],
                    mybir.ActivationFunctionType.Exp,
                    scale=1.0 / TAU,
                    accum_out=st["pos_" + sfx[m]][:, col],
                )

            def emit_hnm(rc, m, sim, g):
                # One DVE pass packs relu_sum + PACK_C*count into the accum
                # (host unpacks). ACT path (engine balance) computes true
                # relu sums + a cheap 4x DVE count on the bf16 out.
                col = slice(rc * NG + g, rc * NG + g + 1)
                r_t = rscr.tile([128, GROUP_COLS], BF16, tag="r", name="r")
                if (rc, m) in ACT_RELU_PAIRS:
                    nc.scalar.activation(
                        r_t[:],
                        sim[:],
                        mybir.ActivationFunctionType.Relu,
                        bias=neg_margin[:],
                        accum_out=st["rsum_" + sfx[m]][:, col],
                    )
                    g_t = gscr.tile([128, GROUP_COLS], BF16, tag="g", name="g")
                    nc.vector.tensor_scalar(
                        g_t[:],
                        r_t[:],
                        0.0,
                        None,
                        mybir.AluOpType.is_gt,
                        mybir.AluOpType.add,
                        accum_out=st["cnt_" + sfx[m]][:, col],
                    )
                else:
                    nc.vector._custom_dve(
                        PACKED_OP,
                        out=r_t[:],
                        in0=sim[:],
                        s0=MARGIN,
                        s1=PACK_C,
                        accum_out=st["rsum_" + sfx[m]][:, col],
                    )

            def body():
                for rc in range(NRC):
                    lhsT = ut_s[:, rc * 128 : (rc + 1) * 128]
                    for m in (0, 1):
                        for g in range(NG):
                            sim = emit_mm(lhsT, m, g)
                            emit_exp(rc, m, sim, g)
                            emit_hnm(rc, m, sim, g)

            if DYN_REPS > 0:
                with tc.For_i(0, DYN_REPS, 1):
                    body()
            else:
                for _rep in range(REPS):
                    body()

            for name, dram in outs.items():
                nc.sync.dma_start(dram[:], st[name][:])

    nc.compile()
    return nc


def _get_built():
    global _BUILT
    if _BUILT is None:
        _BUILT = _build_bass()
    return _BUILT


def gather_partials(results):
    """Combine per-core outputs into per-row [BS] vectors and unpack the
    packed relu/count accumulators."""

    def gather(name):
        out = np.zeros(BS, dtype=np.float64)
        for k in range(8):
            rg = k // C
            arr = results[k][name].astype(np.float64)  # [128, NRC*NG]
            blk = arr.T.reshape(NRC, NG, 128).sum(axis=1).reshape(ROWS)
            out[rg * ROWS : (rg + 1) * ROWS] += blk
        return out

    pos_A, pos_B = gather("pos_a"), gather("pos_b")
    rsum_A, rsum_B = gather("rsum_a"), gather("rsum_b")
    cnt_A, cnt_B = gather("cnt_a"), gather("cnt_b")

    # Unpack relu_sum + PACK_C*count for chunks handled by the packed DVE op.
    rcs = np.arange(BS) % ROWS // 128
    for rsum, cnt, m in ((rsum_A, cnt_A, 0), (rsum_B, cnt_B, 1)):
        for rc in range(NRC):
            if (rc, m) in ACT_RELU_PAIRS:
                continue
            rows = rcs == rc
            packed = rsum[rows]
            c = np.floor(packed / PACK_C + 0.25)
            rsum[rows] = packed - PACK_C * c
            cnt[rows] = c
    return pos_A, pos_B, rsum_A, rsum_B, cnt_A, cnt_B


def _l2norm(x):
    n = np.linalg.norm(x.astype(np.float64), axis=1, keepdims=True)
    return (x.astype(np.float64) / np.maximum(n, 1e-12)).astype(np.float32)


def _round_f32r(x):
    """Round fp32 to the PE's float32r representation: the exactly-split
    sum of two bf16s (hi + lo)."""
    import ml_dtypes

    hi = x.astype(ml_dtypes.bfloat16).astype(np.float32)
    lo = (x - hi).astype(ml_dtypes.bfloat16).astype(np.float32)
    return hi + lo


def make_in_maps(u, a, b):
    """Shard l2-normalized fp32 inputs into the 8 per-core input maps."""
    u = _round_f32r(u)
    a = _round_f32r(a)
    b = _round_f32r(b)
    in_maps = []
    for k in range(8):
        rg, cg = k // C, k % C
        in_maps.append(
            {
                "ut": np.ascontiguousarray(u[rg * ROWS : (rg + 1) * ROWS].T),
                "at": np.ascontiguousarray(a[cg * COLS : (cg + 1) * COLS].T),
                "bt": np.ascontiguousarray(b[cg * COLS : (cg + 1) * COLS].T),
            }
        )
    return in_maps


def kernel(user_interest, reg_A_emb, reg_B_emb):
    global LAST_RESULTS
    u = _round_f32r(_l2norm(np.asarray(user_interest, dtype=np.float32)))
    a = _round_f32r(_l2norm(np.asarray(reg_A_emb, dtype=np.float32)))
    b = _round_f32r(_l2norm(np.asarray(reg_B_emb, dtype=np.float32)))

    in_maps = make_in_maps(u, a, b)

    nc = _get_built()
    res = run_bass_kernel_spmd(nc, in_maps, list(range(8)), trace=TRACE)
    LAST_RESULTS = res

    pos_A, pos_B, rsum_A, rsum_B, cnt_A, cnt_B = gather_partials(res.results)


    # Remove the diagonal contribution from the HNM sums (device included it).
    u64, a64, b64 = u.astype(np.float64), a.astype(np.float64), b.astype(np.float64)
    d_A = np.sum(u64 * a64, axis=1)
    d_B = np.sum(u64 * b64, axis=1)
    rsum_A -= np.maximum(d_A - MARGIN, 0.0)
    rsum_B -= np.maximum(d_B - MARGIN, 0.0)
    cnt_A -= (d_A > MARGIN).astype(np.float64)
    cnt_B -= (d_B > MARGIN).astype(np.float64)

    denom = np.maximum(pos_A + pos_B, 1e-9)
    loss_A = -np.mean(np.log(pos_A / denom))
    loss_B = -np.mean(np.log(pos_B / denom))
    base_loss = (loss_A + loss_B) / 2.0

    def hnm(rsum, cnt):
        has = cnt > 0.5
        n_rows = np.count_nonzero(has)
        if n_rows == 0:
            return 0.0
        total = np.sum(rsum[has] + MARGIN * cnt[has])
        return total / n_rows

    weighted_hard = 0.5 * hnm(rsum_A, cnt_A) + 1.0 * hnm(rsum_B, cnt_B)
    total = base_loss + (
        HARD_NEG_WEIGHT * weighted_hard if abs(weighted_hard) > 1e-9 else 0.0
    )
    return np.float32(total)



# revision 15
# speedup vs baseline: 3.2193x; 3.2193x over previous
"""CrossDomainInterestLoss on 8 Trainium2 NeuronCores.

Strategy (hardcoded for bs=4096, dim=128):
  - Host: l2-normalize u/a/b (fp32), pre-transpose to [dim, rows], shard
    rows of u 4-way and cols (negatives) 2-way -> 8 cores in a 4x2 grid.
  - Device (SPMD): per core, 16 tiles of sim [128, 2048] (8 row-chunks x
    2 negative matrices), each produced by 4 PE matmuls into PSUM fp32.
    Three per-row reductions are needed per tile: pos = sum exp(sim/tau),
    rsum ~ sum relu(sim - margin), cnt = #(sim > margin).
    Engine split exploits DVE fast modes (bf16/u16 SBUF tensor_scalar
    runs 4 elem/cycle; fp32 PSUM runs 1x; ACT is always 1x):
      * route A (14 tiles): ACT computes exp from PSUM (pos via free
        accum_out) writing bf16 e to SBUF; DVE then runs two 4x passes
        on the u16 bit-view of e: bits are ~256*log2(e), so
        max(qA*bits, cA) with accum gives the relu sum (fast-log trick,
        per-entry error ~1.5e-3 only on hard negatives) and
        is_gt(bits, TA) with accum gives the count.
      * route D (2 tiles, rc=7): DVE computes fast-exp bits
        round(SD*sim + BD) from PSUM fp32 (1x) into u16; the bf16
        bit-view IS exp(sim/tau) to ~4% per entry (unbiased), so a 4x
        copy-accum gives pos, and rsum/cnt from the bits are exact
        linear functions of sim (+-1e-4).
    This balances ACT (~26us) and DVE (~27us) instead of the previous
    ACT 32us / DVE 38us split.
  - Host: sum shard partials, remove accum pedestals and the diagonal
    contribution analytically, then apply the exact reference formula.
"""

import math

import numpy as np

import concourse.bass as bass
import concourse.mybir as mybir
from concourse import bacc, tile
from concourse.bass_utils import run_bass_kernel_spmd

TAU = 0.05
HARD_NEG_WEIGHT = 0.5
MARGIN = 0.3
BS = 4096
DIM = 128

R, C = 4, 2           # row-groups x col-groups = 8 cores
ROWS = BS // R        # u rows per core (1024)
COLS = BS // C        # negative rows per core per matrix (2048)
NRC = ROWS // 128     # 128-row chunks per core (8)
NMM = COLS // 512     # matmuls per [128, COLS] tile (4)

F32 = mybir.dt.float32
F32R = mybir.dt.float32r
BF16 = mybir.dt.bfloat16
U16 = mybir.dt.uint16

# Row-chunks handled by route D (DVE fast-exp) instead of route A (ACT exp).
# Both matrices of a chunk share the route so each row's pos_A/pos_B are
# uniformly exact or uniformly fast (log-ratio errors then mostly cancel).
RD_RC = (7,)

# Bit-trick constants. bf16 (1/8/7 layout) bits J of a positive value x
# satisfy J/128 ~= log2(x) + 127 - sigma with |err| <= 0.043 (sigma tuned
# to zero the mean error for mantissa ~ U[1,2)).
LN2 = math.log(2.0)
SIGMA = 0.0573
S_D = 128.0 / (TAU * LN2)          # fast-exp scale: bits = S_D*sim + B_D
B_D = 128.0 * (127.0 - SIGMA)
Q_D = 1.0 / S_D                     # inverse map: sim = Q_D*bits - B_D/S_D
T_D = S_D * MARGIN + B_D            # route-D threshold in bit space (exact)
Q_A = TAU * LN2 / 128.0             # fast-log: sim ~= Q_A*bits - K_A
K_A = TAU * LN2 * (127.0 - SIGMA)


def _exact_bit_threshold():
    """Count threshold for route A: sim > margin <=> e > e* <=> bits(bf16(e))
    above the bf16 bin containing e*, choosing the bin edge with the smaller
    rounding-window bias."""
    import ml_dtypes

    e_star = math.exp(MARGIN / TAU)
    eb = np.float32(e_star).astype(ml_dtypes.bfloat16)
    j = int(eb.view(np.uint16))
    v = float(eb)
    lo = (v + float(np.uint16(j - 1).view(ml_dtypes.bfloat16))) / 2
    hi = (float(np.uint16(j + 1).view(ml_dtypes.bfloat16)) + v) / 2
    return j - 0.5 if (e_star - lo) < (hi - e_star) else j + 0.5


T_A = _exact_bit_threshold()
# Effective sim threshold the route-A bit compare/clamp encodes (~0.29986,
# not exactly MARGIN): host folds the difference back in via the counts.
THR_A = Q_A * T_A - K_A

_BUILT = None
LAST_RESULTS = None  # BassKernelResults of the last run (for profiling)
TRACE = False
REPS = 1  # unrolled repetitions of the whole compute (wall-clock slope timing)
DYN_REPS = 0  # if > 0, wrap the compute in a For_i with this trip count


def _build_bass():
    nc = bacc.Bacc()

    # float32r: fp32 pre-rounded on the host to the PE's two-bf16 split so
    # matmuls stream at 1 cyc/col instead of fp32's 4.
    ut = nc.dram_tensor("ut", [DIM, ROWS], F32R, kind="ExternalInput")
    at = nc.dram_tensor("at", [DIM, COLS], F32R, kind="ExternalInput")
    bt = nc.dram_tensor("bt", [DIM, COLS], F32R, kind="ExternalInput")

    outs = {}
    for name in ("pos_a", "pos_b", "rsum_a", "rsum_b", "cnt_a", "cnt_b"):
        outs[name] = nc.dram_tensor(name, [128, NRC], F32, kind="ExternalOutput")

    with tile.TileContext(nc) as tc:
        with (
            tc.tile_pool(name="ops", bufs=1) as ops,
            tc.tile_pool(name="stats", bufs=1) as stats,
            tc.tile_pool(name="ebuf", bufs=4) as ebuf,
            tc.tile_pool(name="junk", bufs=2) as junk,
            tc.tile_pool(
                name="psum", bufs=2, space=bass.MemorySpace.PSUM
            ) as psum,
        ):
            ut_s = ops.tile([DIM, ROWS], F32R, tag="ut")
            at_s = ops.tile([DIM, COLS], F32R, tag="at")
            bt_s = ops.tile([DIM, COLS], F32R, tag="bt")
            # Loads split across the SP HWDGE path and the gpsimd SWDGE path
            # so ut and the first at half land in parallel.
            half = COLS // 2
            nc.gpsimd.dma_start(ut_s[:], ut[:])
            nc.sync.dma_start(at_s[:, :512], at[:, :512])
            nc.sync.dma_start(at_s[:, 512:half], at[:, 512:half])
            nc.sync.dma_start(at_s[:, half:], at[:, half:])
            nc.sync.dma_start(bt_s[:, :half], bt[:, :half])
            nc.sync.dma_start(bt_s[:, half:], bt[:, half:])

            st = {n: stats.tile([128, NRC], F32, tag=n, name=n) for n in outs}
            # Dummy 1-element exp as the first ACT instruction: the compiler
            # inserts LoadActFuncSet right before it, so the ~1.3us table
            # load overlaps the input DMAs instead of the first real exp.
            warm = stats.tile([128, 1], F32, tag="warm", name="warm")
            nc.scalar.activation(
                warm[:],
                nc.const_aps.tensor(0.0, (128, 1), F32),
                mybir.ActivationFunctionType.Exp,
            )
            for n in outs:
                nc.gpsimd.memset(st[n][:], 0.0)
            neg = {0: at_s, 1: bt_s}
            sfx = {0: "a", 1: "b"}

            def body():
                for rc in range(NRC):
                    lhsT = ut_s[:, rc * 128 : (rc + 1) * 128]
                    for m in (0, 1):
                        sim = psum.tile([128, COLS], F32, tag="sim", name="sim")
                        for n in range(NMM):
                            j0 = n * 512
                            nc.tensor.matmul(
                                sim[:, j0 : j0 + 512],
                                lhsT,
                                neg[m][:, j0 : j0 + 512],
                                start=True,
                                stop=True,
                            )
                        col = slice(rc, rc + 1)
                        pos = st["pos_" + sfx[m]][:, col]
                        rsm = st["rsum_" + sfx[m]][:, col]
                        cnt = st["cnt_" + sfx[m]][:, col]
                        et = ebuf.tile([128, COLS], U16, tag="e", name="e")
                        ev = et[:].bitcast(BF16)
                        if rc in RD_RC:
                            # route D: fast-exp bits on DVE (1x from PSUM),
                            # then three 4x passes on the bits.
                            nc.vector.tensor_scalar(
                                et[:], sim[:], S_D, B_D,
                                mybir.AluOpType.mult, mybir.AluOpType.add,
                            )
                            jp = junk.tile([128, COLS], BF16, tag="j", name="j")
                            nc.vector.tensor_scalar(
                                jp[:], ev, 0.0, None,
                                mybir.AluOpType.add, mybir.AluOpType.add,
                                accum_out=pos,
                            )
                            # accum reduces post-op0 values with op1, so this
                            # yields sum max(bits, T_D) (+ a fixed offset the
                            # host removes via one-time calibration).
                            jr = junk.tile([128, COLS], BF16, tag="j", name="j")
                            nc.vector.tensor_scalar(
                                jr[:], et[:], T_D, -T_D,
                                mybir.AluOpType.max, mybir.AluOpType.add,
                                accum_out=rsm,
                            )
                            jc = junk.tile([128, COLS], BF16, tag="j", name="j")
                            nc.vector.tensor_scalar(
                                jc[:], et[:], T_D, None,
                                mybir.AluOpType.is_gt, mybir.AluOpType.add,
                                accum_out=cnt,
                            )
                        else:
                            # route A: ACT exp (pos via accum), then two 4x
                            # DVE passes on the bit-view of e.
                            nc.scalar.activation(
                                ev,
                                sim[:],
                                mybir.ActivationFunctionType.Exp,
                                scale=1.0 / TAU,
                                accum_out=pos,
                            )
                            jr = junk.tile([128, COLS], BF16, tag="j", name="j")
                            nc.vector.tensor_scalar(
                                jr[:], et[:], T_A, -T_A,
                                mybir.AluOpType.max, mybir.AluOpType.add,
                                accum_out=rsm,
                            )
                            jc = junk.tile([128, COLS], BF16, tag="j", name="j")
                            nc.vector.tensor_scalar(
                                jc[:], et[:], T_A, None,
                                mybir.AluOpType.is_gt, mybir.AluOpType.add,
                                accum_out=cnt,
                            )

            if DYN_REPS > 0:
                with tc.For_i(0, DYN_REPS, 1):
                    body()
            else:
                for _rep in range(REPS):
                    body()

            for name, dram in outs.items():
                nc.sync.dma_start(dram[:], st[name][:])

    nc.compile()
    return nc


def _get_built():
    global _BUILT
    if _BUILT is None:
        _BUILT = _build_bass()
    return _BUILT


def gather_partials(results, rsum_cal):
    """Combine per-core outputs into per-row [BS] vectors. The device rsum
    accums are sum max(bits, T) + fixed_offset in bit units; rsum_cal holds
    the calibrated (offset + COLS*T) pedestal per core/partition/chunk.
    Scale to sim units per route and shift route-A's slightly-off bit
    threshold back to MARGIN via the counts."""

    def gather(name, cal=None):
        out = np.zeros(BS, dtype=np.float64)
        for k in range(8):
            rg = k // C
            arr = results[k][name].astype(np.float64)  # [128, NRC]
            if cal is not None:
                arr = arr - cal[name][k]
            out[rg * ROWS : (rg + 1) * ROWS] += arr.T.reshape(ROWS)
        return out

    pos_A, pos_B = gather("pos_a"), gather("pos_b")
    rsum_A = gather("rsum_a", rsum_cal)
    rsum_B = gather("rsum_b", rsum_cal)
    cnt_A, cnt_B = gather("cnt_a"), gather("cnt_b")

    rcs = (np.arange(BS) % ROWS) // 128
    is_d = np.isin(rcs, RD_RC)
    q = np.where(is_d, Q_D, Q_A)
    off = np.where(is_d, 0.0, MARGIN - THR_A)
    rsum_A = q * rsum_A - off * cnt_A
    rsum_B = q * rsum_B - off * cnt_B
    return pos_A, pos_B, rsum_A, rsum_B, cnt_A, cnt_B


def _l2norm(x):
    n = np.linalg.norm(x.astype(np.float64), axis=1, keepdims=True)
    return (x.astype(np.float64) / np.maximum(n, 1e-12)).astype(np.float32)


def _round_f32r(x):
    """Round fp32 to the PE's float32r representation: the exactly-split
    sum of two bf16s (hi + lo)."""
    import ml_dtypes

    hi = x.astype(ml_dtypes.bfloat16).astype(np.float32)
    lo = (x - hi).astype(ml_dtypes.bfloat16).astype(np.float32)
    return hi + lo


def make_in_maps(u, a, b):
    """Shard l2-normalized fp32 inputs into the 8 per-core input maps."""
    u = _round_f32r(u)
    a = _round_f32r(a)
    b = _round_f32r(b)
    in_maps = []
    for k in range(8):
        rg, cg = k // C, k % C
        in_maps.append(
            {
                "ut": np.ascontiguousarray(u[rg * ROWS : (rg + 1) * ROWS].T),
                "at": np.ascontiguousarray(a[cg * COLS : (cg + 1) * COLS].T),
                "bt": np.ascontiguousarray(b[cg * COLS : (cg + 1) * COLS].T),
            }
        )
    return in_maps


_RSUM_CAL = None


def _host_bits(u, negs):
    """Exact host model of the per-core bit tensors the rsum pass reduces."""
    import ml_dtypes

    simf = (u.astype(np.float64) @ negs.astype(np.float64).T).astype(np.float32)
    e = np.exp(simf / np.float32(TAU))
    J = e.astype(ml_dtypes.bfloat16).view(np.uint16).astype(np.float64)
    I = np.round(np.float32(S_D) * simf + np.float32(B_D)).astype(np.float64)
    return J, I


def _calibrate(nc):
    """Run the kernel once on synthetic inputs and extract the constant
    additive offsets of the rsum accumulators (device accum minus the exact
    sum max(bits, T)), which are instruction-layout constants."""
    rng = np.random.default_rng(123)
    u = _l2norm(rng.standard_normal((BS, DIM)).astype(np.float32))
    a = _l2norm(rng.standard_normal((BS, DIM)).astype(np.float32))
    b = _l2norm(rng.standard_normal((BS, DIM)).astype(np.float32))
    u, a, b = _round_f32r(u), _round_f32r(a), _round_f32r(b)
    res = run_bass_kernel_spmd(nc, make_in_maps(u, a, b), list(range(8)))

    cal = {"rsum_a": [], "rsum_b": []}
    for k in range(8):
        rg, cg = k // C, k % C
        us = u[rg * ROWS : (rg + 1) * ROWS]
        for name, negs in (("rsum_a", a), ("rsum_b", b)):
            ns = negs[cg * COLS : (cg + 1) * COLS]
            J, I = _host_bits(us, ns)
            relu_bits = np.empty((128, NRC))
            for rc in range(NRC):
                bits = I if rc in RD_RC else J
                t = T_D if rc in RD_RC else T_A
                relu_bits[:, rc] = np.maximum(
                    bits[rc * 128 : (rc + 1) * 128] - t, 0.0
                ).sum(axis=1)
            cal[name].append(res.results[k][name].astype(np.float64) - relu_bits)
    return cal


def kernel(user_interest, reg_A_emb, reg_B_emb):
    global LAST_RESULTS, _RSUM_CAL
    u = _round_f32r(_l2norm(np.asarray(user_interest, dtype=np.float32)))
    a = _round_f32r(_l2norm(np.asarray(reg_A_emb, dtype=np.float32)))
    b = _round_f32r(_l2norm(np.asarray(reg_B_emb, dtype=np.float32)))

    in_maps = make_in_maps(u, a, b)

    nc = _get_built()
    if _RSUM_CAL is None:
        _RSUM_CAL = _calibrate(nc)
    res = run_bass_kernel_spmd(nc, in_maps, list(range(8)), trace=TRACE)
    LAST_RESULTS = res

    pos_A, pos_B, rsum_A, rsum_B, cnt_A, cnt_B = gather_partials(
        res.results, _RSUM_CAL
    )

    # Remove the diagonal contribution from the HNM sums (device included it).
    u64, a64, b64 = u.astype(np.float64), a.astype(np.float64), b.astype(np.float64)
    d_A = np.sum(u64 * a64, axis=1)
    d_B = np.sum(u64 * b64, axis=1)
    rsum_A -= np.maximum(d_A - MARGIN, 0.0)
    rsum_B -= np.maximum(d_B - MARGIN, 0.0)
    cnt_A -= (d_A > MARGIN).astype(np.float64)
    cnt_B -= (d_B > MARGIN).astype(np.float64)

    denom = np.maximum(pos_A + pos_B, 1e-9)
    loss_A = -np.mean(np.log(pos_A / denom))
    loss_B = -np.mean(np.log(pos_B / denom))
    base_loss = (loss_A + loss_B) / 2.0

    def hnm(rsum, cnt):
        has = cnt > 0.5
        n_rows = np.count_nonzero(has)
        if n_rows == 0:
            return 0.0
        total = np.sum(rsum[has] + MARGIN * cnt[has])
        return total / n_rows

    weighted_hard = 0.5 * hnm(rsum_A, cnt_A) + 1.0 * hnm(rsum_B, cnt_B)
    total = base_loss + (
        HARD_NEG_WEIGHT * weighted_hard if abs(weighted_hard) > 1e-9 else 0.0
    )
    return np.float32(total)
